# revision 5
# baseline (speedup 1.0000x reference)
"""Gated channel-attention (B=32, C=512, T=1024) on 8 Trainium2 NeuronCores.

Math per batch b (torch/jax layout):
    q = gq * (x^T @ Wq^T + bq)          [T, C]
    k = gk * (x^T @ Wk^T + bk)
    v = gv * (x^T @ Wv^T + bv)
    energy = q^T @ k                    [C, C]   (contraction over T)
    attn   = softmax(energy / sqrt(C))  (rows)
    out    = attn @ v^T                 [C, T]

Sharding: pure data-parallel over batch B — 4 batches per core, no
collectives.

End-to-end wall time of kernel() is dominated by the axon tunnel
(~40 MiB/s up, ~27 MiB/s down, full duplex), not device compute
(~0.1 ms/batch/core). So the host path is built around minimizing and
overlapping transferred bytes:
  - x is shipped as bf16 (the device matmuls consume bf16 anyway).
  - gates are shipped as uint8 (g8 = round(g*255)); the dequant 1/255 is
    folded into the Q/K/V weights and biases host-side, so the device just
    converts u8 -> bf16 (exact for 0..255) and proceeds unchanged:
      g8 * (x @ (W/255) + b/255) == g * (x @ W + b).
  - the output returns as bf16 and is upcast on host.
  - weights/biases and the output-donation zero buffers live on device
    across calls (re-uploaded only if the weight bytes change).
  - work is split into NCH chunks of NBC batches/core, pipelined:
    chunk i+1 uploads while chunk i executes and downloads (the tunnel is
    full duplex, so downloads are free until uploads finish).

Device kernel layout (per 128-partition tiles):
  - x, gates arrive channel-major [C, T], exactly what the projection
    matmuls and the gating want.
  - bias+gate are fused in one DVE scalar_tensor_tensor (PSUM -> SBUF),
    emitting bf16.
  - q, k are transposed to [T, C] with PE transpose-mode (bf16).
  - energy is computed transposed ([d, c]) so exp(d-major) feeds the
    attn@v matmul with no further transposes; softmax normalization is
    folded into the output as U[c,t] * (1/Z[c]), with Z computed by a
    ones-vector matmul. Logits are ~|x|<=1.5 so exp needs no max-shift.
"""

import hashlib
import math
from concurrent.futures import ThreadPoolExecutor

import numpy as np

B, C, T = 32, 512, 1024
P = 128
CT = C // P          # 4 channel tiles
TT = T // P          # 8 time tiles
NH = T // 512        # 2 halves of the free dim for 512-wide matmuls
SCALE = 1.0 / math.sqrt(512.0)

NBC = 1              # batches per core per chunk
NCH = (B // 8) // NBC  # chunks per call

_CACHE = {}


def _patch_tile_drain():
    """This container's walrus rejects instructions carrying more than one
    (two for EventSemaphore) semaphore waits, but Tile attaches every
    required wait to the consuming instruction. Spill excess waits onto
    preceding same-engine NoOps (sequentially equivalent), and re-emit the
    final drain as one drain per wait."""
    import concourse.mybir as mybir
    import concourse.tile as tile_mod
    from bass_rust import ScopedClock

    if getattr(tile_mod.TileContext, "_drain_split_patch", False):
        return

    orig_commit = tile_mod.TileContext._commit_instruction

    def _commit_instruction(self, inst, lazy_reg_writes=True):
        si = getattr(inst, "sync_info", None)
        if si is not None and len(si.on_wait) > 1:
            waits = list(si.on_wait)
            for w in waits[1:]:
                sp = mybir.InstNoOp(
                    name=self.nc.get_next_instruction_name(),
                    engine=inst.engine,
                    sync_info=mybir.SyncInfo(on_wait=[w], on_update=[]),
                    bass_nofuse=True,
                )
                orig_commit(self, sp, lazy_reg_writes)
            inst.sync_info = mybir.SyncInfo(
                on_wait=waits[:1], on_update=list(si.on_update)
            )
        return orig_commit(self, inst, lazy_reg_writes)

    tile_mod.TileContext._commit_instruction = _commit_instruction

    def _drain_and_barrier(self, tick_clock, wait_clock):
        nc = self.nc
        probe = mybir.InstNoOp(name="wait-probe", ins=[], outs=[])
        probe.engine = mybir.EngineType.SP
        wait_clock.add_sem_waits(probe, ScopedClock({None: tick_clock.global_clock}))
        si = probe.sync_info
        waits = list(si.on_wait) if si is not None else []
        assert self.sems is not None
        id2sem = {h.num: h for h in self.sems.allocated().values()}
        if not waits:
            nc.sync.drain()
        for w in waits:
            assert w.sync_type == "semaphore", w
            nc.sync.drain().wait_op(id2sem[w.id], w.wait_value, "sem-ge")
        nc.all_engine_barrier()
        popped = nc._tile_sem_poison_stack.pop()
        assert popped is self._sem_poison
        nc.clear_and_free_semaphores(list(self.sems.allocated().values()))
        nc.all_engine_barrier()

    tile_mod.TileContext._drain_and_barrier = _drain_and_barrier
    tile_mod.TileContext._drain_split_patch = True


def _build(nb):
    import concourse.bass as bass
    import concourse.mybir as mybir
    import concourse.tile as tile
    from concourse.masks import make_identity

    _patch_tile_drain()

    f32 = mybir.dt.float32
    bf16 = mybir.dt.bfloat16
    u8 = mybir.dt.uint8
    add = mybir.AluOpType.add
    mult = mybir.AluOpType.mult

    nc = bass.Bass()
    x_d = nc.declare_dram_parameter("x", [nb, C, T], bf16, isOutput=False)
    g_d = {
        "q": nc.declare_dram_parameter("gq", [nb, C, T], u8, isOutput=False),
        "k": nc.declare_dram_parameter("gk", [nb, C, T], u8, isOutput=False),
        "v": nc.declare_dram_parameter("gv", [nb, C, T], u8, isOutput=False),
    }
    # weights host-packed as W^T/255 (bf16); biases as [P, CT] f32 of b/255
    wt_d = {
        "q": nc.declare_dram_parameter("wqt", [C, C], bf16, isOutput=False),
        "k": nc.declare_dram_parameter("wkt", [C, C], bf16, isOutput=False),
        "v": nc.declare_dram_parameter("wvt", [C, C], bf16, isOutput=False),
    }
    b_d = {
        "q": nc.declare_dram_parameter("bq", [P, CT], f32, isOutput=False),
        "k": nc.declare_dram_parameter("bk", [P, CT], f32, isOutput=False),
        "v": nc.declare_dram_parameter("bv", [P, CT], f32, isOutput=False),
    }
    out_d = nc.declare_dram_parameter("out", [nb, C, T], bf16, isOutput=True)

    with tile.TileContext(nc) as tc:
        from contextlib import ExitStack

        with ExitStack() as ctx:
            const = ctx.enter_context(tc.tile_pool(name="const", bufs=1))
            xb_p = ctx.enter_context(tc.tile_pool(name="xb", bufs=5))
            g8_p = ctx.enter_context(tc.tile_pool(name="g8", bufs=4))
            gate_p = ctx.enter_context(tc.tile_pool(name="gate", bufs=6))
            qkc_p = ctx.enter_context(tc.tile_pool(name="qkc", bufs=10))
            vb_p = ctx.enter_context(tc.tile_pool(name="vb", bufs=5))
            qkt_p = ctx.enter_context(tc.tile_pool(name="qkt", bufs=18))
            exp_p = ctx.enter_context(tc.tile_pool(name="expp", bufs=8))
            rz_p = ctx.enter_context(tc.tile_pool(name="rz", bufs=8))
            out_p = ctx.enter_context(tc.tile_pool(name="outs", bufs=4))
            pmm = ctx.enter_context(tc.tile_pool(name="pmm", bufs=4, space="PSUM"))
            ptp = ctx.enter_context(tc.tile_pool(name="ptp", bufs=3, space="PSUM"))
            pz = ctx.enter_context(tc.tile_pool(name="pz", bufs=1, space="PSUM"))

            wt = {}
            bias = {}

            def load_consts(p):
                for ci in range(CT):
                    w = const.tile([P, C], bf16, tag=f"wt_{p}{ci}")
                    nc.sync.dma_start(w[:], wt_d[p][ci * P:(ci + 1) * P, :])
                    wt[(p, ci)] = w
                bt = const.tile([P, CT], f32, tag=f"b_{p}")
                nc.sync.dma_start(bt[:], b_d[p][:])
                for di in range(CT):
                    bias[(p, di)] = bt[:, di:di + 1]

            # critical-path order: batch-0 x and q-weights first; k/v weights
            # loaded behind them inside the first batch
            load_consts("q")
            ident = const.tile([P, P], bf16, tag="ident")
            make_identity(nc, ident[:])
            ones = const.tile([P, 1], bf16, tag="ones")
            nc.gpsimd.memset(ones[:], 1.0)

            for bi in range(nb):
                # ---- load x (channel-major, contiguous, bf16) ----
                xb = []
                for ci in range(CT):
                    c_ = xb_p.tile([P, T], bf16, tag="xb")
                    nc.sync.dma_start(c_[:], x_d[bi, ci * P:(ci + 1) * P, :])
                    xb.append(c_)
                if bi == 0:
                    load_consts("k")
                    load_consts("v")

                # ---- projections + fused bias+gate (bf16 matmul) ----
                def project(p):
                    pool = vb_p if p == "v" else qkc_p
                    dtiles = []
                    for di in range(CT):
                        g8 = g8_p.tile([P, T], u8, tag="g8")
                        nc.sync.dma_start(g8[:], g_d[p][bi, di * P:(di + 1) * P, :])
                        g = gate_p.tile([P, T], bf16, tag="gate")
                        # u8 -> bf16 (integers 0..255, exact); keep ScalarE
                        # exp-only and DVE for the fused bias+gate
                        nc.gpsimd.tensor_copy(g[:], g8[:])
                        dst = pool.tile([P, T], bf16, tag="vb" if p == "v" else "qkc")
                        for th in range(NH):
                            ps = pmm.tile([P, 512], f32, tag="pmm")
                            sl = slice(th * 512, (th + 1) * 512)
                            for ci in range(CT):
                                nc.tensor.matmul(
                                    ps[:],
                                    wt[(p, ci)][:, di * P:(di + 1) * P],
                                    xb[ci][:, sl],
                                    start=(ci == 0),
                                    stop=(ci == CT - 1),
                                )
                            # (proj + bias) * gate  -> bf16
                            nc.vector.scalar_tensor_tensor(
                                dst[:, sl], ps[:], bias[(p, di)], g[:, sl],
                                op0=add, op1=mult,
                            )
                        dtiles.append(dst)
                    return dtiles

                def transpose(dtiles):
                    ttiles = []
                    for ti in range(TT):
                        dst = qkt_p.tile([P, C], bf16, tag="qkt")
                        tp = ptp.tile([P, C], bf16, tag="ptp")
                        for di in range(CT):
                            nc.tensor.transpose(
                                tp[:, di * P:(di + 1) * P],
                                dtiles[di][:, ti * P:(ti + 1) * P],
                                ident[:],
                            )
                        nc.vector.tensor_copy(dst[:], tp[:])
                        ttiles.append(dst)
                    return ttiles

                dests = {}
                tmaj = {}
                dests["q"] = project("q")
                tmaj["q"] = transpose(dests["q"])
                dests["k"] = project("k")
                tmaj["k"] = transpose(dests["k"])
                dests["v"] = project("v")

                # ---- energy^T [d, c] and exp ----
                expT = []
                for di in range(CT):
                    ps = pmm.tile([P, C], f32, tag="pmm")
                    for ti in range(TT):
                        nc.tensor.matmul(
                            ps[:],
                            tmaj["k"][ti][:, di * P:(di + 1) * P],
                            tmaj["q"][ti][:],
                            start=(ti == 0),
                            stop=(ti == TT - 1),
                        )
                    e = exp_p.tile([P, C], bf16, tag="expp")
                    nc.scalar.activation(
                        e[:], ps[:], mybir.ActivationFunctionType.Exp, scale=SCALE
                    )
                    expT.append(e)

                # ---- Z[c] = sum_d exp^T[d, c] via ones matmul; 1/Z ----
                rz = []
                for cj in range(CT):
                    z = pz.tile([P, 1], f32, tag="pz")
                    for di in range(CT):
                        nc.tensor.matmul(
                            z[:],
                            expT[di][:, cj * P:(cj + 1) * P],
                            ones[:],
                            start=(di == 0),
                            stop=(di == CT - 1),
                        )
                    r = rz_p.tile([P, 1], f32, tag="rz")
                    nc.vector.reciprocal(r[:], z[:])
                    rz.append(r)

                # ---- U[c, t] = exp^T.T @ v ; out = U / Z ----
                for cj in range(CT):
                    for th in range(NH):
                        ps = pmm.tile([P, 512], f32, tag="pmm")
                        sl = slice(th * 512, (th + 1) * 512)
                        for di in range(CT):
                            nc.tensor.matmul(
                                ps[:],
                                expT[di][:, cj * P:(cj + 1) * P],
                                dests["v"][di][:, sl],
                                start=(di == 0),
                                stop=(di == CT - 1),
                            )
                        o = out_p.tile([P, 512], bf16, tag="outs")
                        nc.vector.tensor_scalar_mul(o[:], ps[:], rz[cj][:])
                        nc.sync.dma_start(
                            out_d[bi, cj * P:(cj + 1) * P, sl], o[:]
                        )
    return nc


def _runtime():
    rt = _CACHE.get("rt")
    if rt is not None:
        return rt
    import jax
    import ml_dtypes
    from jax.sharding import Mesh, NamedSharding, PartitionSpec

    try:
        from jax.experimental.shard_map import shard_map
    except ImportError:
        from jax.shard_map import shard_map
    import concourse.mybir as mybir
    from concourse.bass2jax import (
        _bass_exec_p,
        install_neuronx_cc_hook,
        partition_id_tensor,
    )

    nc = _build(NBC)
    install_neuronx_cc_hook()
    pname = nc.partition_id_tensor.name if nc.partition_id_tensor else None
    in_names, out_names, out_avals = [], [], []
    for alloc in nc.m.functions[0].allocations:
        if not isinstance(alloc, mybir.MemoryLocationSet):
            continue
        name = alloc.memorylocations[0].name
        if alloc.kind == "ExternalInput":
            if name != pname:
                in_names.append(name)
        elif alloc.kind == "ExternalOutput":
            out_names.append(name)
            out_avals.append(
                jax.core.ShapedArray(
                    tuple(alloc.tensor_shape), mybir.dt.np(alloc.dtype)
                )
            )
    all_names = tuple(in_names) + tuple(out_names)
    if pname:
        all_names += (pname,)

    def body(*args):
        operands = list(args)
        if pname:
            operands.append(partition_id_tensor())
        return tuple(
            _bass_exec_p.bind(
                *operands,
                out_avals=tuple(out_avals),
                in_names=all_names,
                out_names=tuple(out_names),
                lowering_input_output_aliases=(),
                sim_require_finite=True,
                sim_require_nnan=True,
                nc=nc,
            )
        )

    mesh = Mesh(np.asarray(jax.devices()[:8]), ("core",))
    nops = len(in_names) + len(out_names)
    f = jax.jit(
        shard_map(
            body,
            mesh=mesh,
            in_specs=(PartitionSpec("core"),) * nops,
            out_specs=(PartitionSpec("core"),) * len(out_names),
            check_rep=False,
        )
    )
    sh = NamedSharding(mesh, PartitionSpec("core"))
    zeros = [
        jax.device_put(np.zeros((8 * a.shape[0], *a.shape[1:]), a.dtype), sh)
        for a in out_avals
    ]
    rt = dict(
        jax=jax,
        f=f,
        sh=sh,
        in_names=in_names,
        zeros=zeros,
        bf16=ml_dtypes.bfloat16,
        consts=None,
        consts_key=None,
        pool=ThreadPoolExecutor(4),
        prep_pool=ThreadPoolExecutor(3),
    )
    _CACHE["rt"] = rt
    return rt


def kernel(x, g_query, g_keys, g_values, Wq, bq, Wk, bk, Wv, bv):
    rt = _runtime()
    jax = rt["jax"]
    sh = rt["sh"]
    bf = rt["bf16"]
    f = rt["f"]

    # device-resident weights; re-upload only if the bytes change
    key = b"".join(
        hashlib.sha1(np.ascontiguousarray(np.asarray(a)).view(np.uint8)).digest()
        for a in (Wq, bq, Wk, bk, Wv, bv)
    )
    if rt["consts_key"] != key:
        s = np.float32(1.0 / 255.0)

        def prep_w(W):
            w = np.ascontiguousarray(
                (np.asarray(W, np.float32).T * s).astype(bf)
            )
            return jax.device_put(np.concatenate([w] * 8, axis=0), sh)

        def prep_b(b):
            br = np.ascontiguousarray(
                (np.asarray(b, np.float32) * s).reshape(CT, P).T
            )
            return jax.device_put(np.concatenate([br] * 8, axis=0), sh)

        consts = {
            "wqt": prep_w(Wq),
            "wkt": prep_w(Wk),
            "wvt": prep_w(Wv),
            "bq": prep_b(bq),
            "bk": prep_b(bk),
            "bv": prep_b(bv),
        }
        jax.block_until_ready(list(consts.values()))
        rt["consts"] = consts
        rt["consts_key"] = key
    consts = rt["consts"]

    x5 = np.asarray(x).reshape(8, NCH, NBC, C, T)
    g5 = {
        "gq": np.asarray(g_query).reshape(8, NCH, NBC, C, T),
        "gk": np.asarray(g_keys).reshape(8, NCH, NBC, C, T),
        "gv": np.asarray(g_values).reshape(8, NCH, NBC, C, T),
    }

    def quant_gate(a):
        t = np.multiply(a, np.float32(255.0), dtype=np.float32)
        np.add(t, np.float32(0.5), out=t)
        return t.astype(np.uint8).reshape(8 * NBC, C, T)

    # host-side cast/quantize runs in threads so it overlaps with the
    # (bandwidth-bound) uploads of earlier chunks
    def prep(j):
        return {
            "x": x5[:, j].astype(bf).reshape(8 * NBC, C, T),
            "gq": quant_gate(g5["gq"][:, j]),
            "gk": quant_gate(g5["gk"][:, j]),
            "gv": quant_gate(g5["gv"][:, j]),
        }

    prep_futs = [rt["prep_pool"].submit(prep, j) for j in range(NCH)]

    res = np.empty((8, NCH, NBC, C, T), np.float32)

    def fetch(j, o):
        arr = np.asarray(o)  # [8*NBC, C, T] bf16
        res[:, j] = arr.reshape(8, NBC, C, T).astype(np.float32)

    futs = []
    for j in range(NCH):
        h = prep_futs[j].result()
        up = {n: jax.device_put(h[n], sh) for n in ("x", "gq", "gk", "gv")}
        args = [up[n] if n in up else consts[n] for n in rt["in_names"]]
        out = f(*args, *rt["zeros"])
        futs.append(rt["pool"].submit(fetch, j, out[0]))
    for fu in futs:
        fu.result()
    return res.reshape(B, C, T)
